# revision 5
# baseline (speedup 1.0000x reference)
"""ArcFace logits kernel for 8 TRN2 NeuronCores (partial-FC tensor parallel).

logits = scale * where(one_hot(labels), cos(arccos(cosine)+m), cosine)
  cosine = normalize(emb) @ normalize(W)   [B=512, C=100000]

Sharding: W columns (and the [B, C] output) split across 8 cores, 12500
columns each; embeddings/labels broadcast. No collectives needed.

Per-core device pipeline, per 500-column chunk:
  - column norms via ones-vector matmul over W^2 (partition-axis reduce)
  - rsqrt broadcast to 128 partitions via K=1 ones matmul
  - main matmul: psum = (64 * emb_norm^T) @ W_raw, accumulated over D
  - margin fixup accumulated into the same psum as a K=n_slot one-hot
    matmul: host passes 0/1 selection masks (pure label-index plumbing),
    the device computes the per-row delta values from W[:, labels]
  - evict: out = psum * rsqrt_bcast  (one DVE tensor_tensor per tile)
"""

import math

import numpy as np

import concourse.bass as bass
import concourse.tile as tile
from concourse import mybir
from concourse.bass_utils import run_bass_kernel_spmd

N_CORES = 8
B = 512          # batch
D = 512          # embed dim
C = 100000       # num classes
CS = C // N_CORES          # 12500 columns per core
CHUNK = 500
DT = D // 128
BT = B // 128
SCALE = 64.0
MARGIN = 0.5
F32 = mybir.dt.float32
AF = mybir.ActivationFunctionType

_MAX_WAITS = 1


def _legalize_waits(nc, max_waits=_MAX_WAITS):
    """Split multi-wait instructions for this toolchain's codegen.

    The pinned neuronxcc rejects instructions carrying more than one sync
    wait ("Too many sync wait commands" in setupSyncWait). Tile's semaphore
    assignment can attach several waits to one instruction (tail drain,
    first matmul of a group). Hoist the overflow onto no-op instructions
    emitted just before, on the same engine — the engine blocks on those
    first, which is semantically identical.
    """
    n = 0
    for fn in nc.m.functions:
        for bb in fn.blocks:
            out = []
            for inst in bb.instructions:
                si = inst.sync_info
                if si is not None and si.on_wait and len(si.on_wait) > max_waits:
                    waits = list(si.on_wait)
                    keep = waits[-max_waits:]
                    over = waits[:-max_waits]
                    for i in range(0, len(over), max_waits):
                        nop = mybir.InstNoOp(
                            name=f"waitsplit_{n}",
                            sync_info=mybir.SyncInfo(
                                on_wait=over[i : i + max_waits], on_update=[]
                            ),
                            bass_nofuse=True,
                            engine=inst.engine,
                        )
                        n += 1
                        out.append(nop)
                    inst.sync_info = mybir.SyncInfo(
                        on_wait=keep, on_update=list(si.on_update or [])
                    )
                out.append(inst)
            bb.instructions[:] = out
    return n


def build(k_fix, cs=CS, sq_bufs=6, w_bufs=3, out_bufs=3, ps_bufs=8):
    nchunk = cs // CHUNK
    nc = bass.Bass("TRN2", target_bir_lowering=False, debug=False, num_devices=N_CORES)
    w_ext = nc.declare_dram_parameter("w", [D, cs], F32, isOutput=False)
    embt_ext = nc.declare_dram_parameter("embT", [D, B], F32, isOutput=False)
    wl_ext = nc.declare_dram_parameter("wl", [D, B], F32, isOutput=False)
    sel_ext = nc.declare_dram_parameter("sel", [nchunk, k_fix, B], F32, isOutput=False)
    oh_ext = nc.declare_dram_parameter("oh", [nchunk, k_fix, CHUNK], F32, isOutput=False)
    onesc_ext = nc.declare_dram_parameter("onesc", [128, 1], F32, isOutput=False)
    onesr_ext = nc.declare_dram_parameter("onesr", [1, 128], F32, isOutput=False)
    out_ext = nc.declare_dram_parameter("out", [B, cs], F32, isOutput=True)

    w_ap = w_ext.ap().rearrange("(a p) c -> p a c", p=128)      # [128, DT, cs]
    et_ap = embt_ext.ap().rearrange("(a p) b -> p a b", p=128)  # [128, DT, B]
    wl_ap = wl_ext.ap().rearrange("(a p) b -> p a b", p=128)
    out_ap = out_ext.ap().rearrange("(a p) c -> p a c", p=128)  # [128, BT, cs]

    cosm = math.cos(MARGIN)
    sinm = math.sin(MARGIN)

    with tile.TileContext(nc) as tc:
        with (
            tc.tile_pool(name="persist", bufs=1) as persist,
            tc.tile_pool(name="ps", bufs=ps_bufs, space="PSUM") as psp,
            tc.tile_pool(name="pro", bufs=3) as pro,
            tc.tile_pool(name="wp", bufs=w_bufs) as wp,
            tc.tile_pool(name="sqp", bufs=sq_bufs) as sqp,
            tc.tile_pool(name="rp", bufs=3) as rp,
            tc.tile_pool(name="op", bufs=out_bufs) as op,
            tc.tile_pool(name="fx", bufs=3) as fx,
        ):
            # ---- persistent tiles
            oc = persist.tile([128, 1], F32)
            nc.sync.dma_start(out=oc[:], in_=onesc_ext.ap())
            orow = persist.tile([1, 128], F32)
            nc.sync.dma_start(out=orow[:], in_=onesr_ext.ap())
            et = persist.tile([128, DT, B], F32)
            nc.sync.dma_start(out=et[:], in_=et_ap)
            e64 = persist.tile([128, DT, B], F32)
            ebc = persist.tile([128, B], F32)
            dpb = persist.tile([k_fix, B], F32)

            # ---- prologue: per-row margin delta from the gathered label
            # columns wl = W[:, labels], plus 64/||emb|| row scaling.
            wlt = pro.tile([128, DT, B], F32, bufs=1)
            nc.sync.dma_start(out=wlt[:], in_=wl_ap)

            p_ne = psp.tile([1, B], F32, tag="ps")
            for d in range(DT):
                sq = pro.tile([128, B], F32)
                nc.scalar.activation(out=sq[:], in_=et[:, d, :], func=AF.Square)
                nc.tensor.matmul(
                    p_ne[:], lhsT=oc[:], rhs=sq[:], start=(d == 0), stop=(d == DT - 1)
                )
            lne = pro.tile([1, B], F32, bufs=1)
            nc.scalar.activation(out=lne[:], in_=p_ne[:], func=AF.Ln)
            rinv_e = pro.tile([1, B], F32, bufs=1)
            nc.scalar.activation(out=rinv_e[:], in_=lne[:], func=AF.Exp, scale=-0.5)

            p_dot = psp.tile([1, B], F32, tag="ps")
            for d in range(DT):
                hd = pro.tile([128, B], F32)
                nc.vector.tensor_mul(hd[:], et[:, d, :], wlt[:, d, :])
                nc.tensor.matmul(
                    p_dot[:], lhsT=oc[:], rhs=hd[:], start=(d == 0), stop=(d == DT - 1)
                )

            p_nw = psp.tile([1, B], F32, tag="ps")
            for d in range(DT):
                sqw = pro.tile([128, B], F32)
                nc.scalar.activation(out=sqw[:], in_=wlt[:, d, :], func=AF.Square)
                nc.tensor.matmul(
                    p_nw[:], lhsT=oc[:], rhs=sqw[:], start=(d == 0), stop=(d == DT - 1)
                )
            lnw = pro.tile([1, B], F32, bufs=1)
            nc.scalar.activation(out=lnw[:], in_=p_nw[:], func=AF.Ln)
            rinv_w = pro.tile([1, B], F32, bufs=1)
            nc.scalar.activation(out=rinv_w[:], in_=lnw[:], func=AF.Exp, scale=-0.5)
            snw = pro.tile([1, B], F32, bufs=1)
            nc.scalar.activation(out=snw[:], in_=p_nw[:], func=AF.Sqrt)

            cos1 = pro.tile([1, B], F32, bufs=1)
            nc.vector.tensor_mul(cos1[:], p_dot[:], rinv_e[:])
            cosl = pro.tile([1, B], F32, bufs=1)
            nc.vector.tensor_mul(cosl[:], cos1[:], rinv_w[:])

            cos2 = pro.tile([1, B], F32)
            nc.scalar.activation(out=cos2[:], in_=cosl[:], func=AF.Square)
            om = pro.tile([1, B], F32)
            nc.vector.tensor_scalar(
                om[:], cos2[:], -1.0, 1.0, mybir.AluOpType.mult, mybir.AluOpType.add
            )
            sn = pro.tile([1, B], F32)
            nc.scalar.activation(out=sn[:], in_=om[:], func=AF.Sqrt)
            # delta64 = 64*(cos(th+m) - cos) = cos*64*(cosm-1) - sin*64*sinm
            t2 = pro.tile([1, B], F32)
            nc.scalar.mul(out=t2[:], in_=cosl[:], mul=SCALE * (cosm - 1.0))
            t3 = pro.tile([1, B], F32)
            nc.scalar.mul(out=t3[:], in_=sn[:], mul=-SCALE * sinm)
            d64 = pro.tile([1, B], F32)
            nc.vector.tensor_add(d64[:], t2[:], t3[:])
            # pre-divide by the rsqrt that the evict multiply will apply at
            # the label column: delta_pre = delta64 * ||w_label||
            dpre = pro.tile([1, B], F32, bufs=1)
            nc.vector.tensor_mul(dpre[:], d64[:], snw[:])

            # broadcast delta_pre to k_fix partitions (K=1 ones matmul)
            p_dpb = psp.tile([k_fix, B], F32, tag="ps")
            nc.tensor.matmul(
                p_dpb[:], lhsT=orow[:, :k_fix], rhs=dpre[:], start=True, stop=True
            )
            nc.scalar.copy(out=dpb[:], in_=p_dpb[:])

            # 64/||emb|| broadcast to 128 partitions, then scale embT
            rinv64 = pro.tile([1, B], F32, bufs=1)
            nc.scalar.mul(out=rinv64[:], in_=rinv_e[:], mul=SCALE)
            p_eb = psp.tile([128, B], F32, tag="ps")
            nc.tensor.matmul(p_eb[:], lhsT=orow[:], rhs=rinv64[:], start=True, stop=True)
            nc.scalar.copy(out=ebc[:], in_=p_eb[:])
            for d in range(DT):
                nc.vector.tensor_mul(e64[:, d, :], et[:, d, :], ebc[:])

            # ---- main loop over 500-column chunks
            for j in range(nchunk):
                wch = wp.tile([128, DT, CHUNK], F32, tag="wch")
                nc.sync.dma_start(
                    out=wch[:], in_=w_ap[:, :, j * CHUNK : (j + 1) * CHUNK]
                )
                selt = fx.tile([k_fix, B], F32, tag="selt")
                nc.sync.dma_start(out=selt[:], in_=sel_ext.ap()[j])
                oht = fx.tile([k_fix, CHUNK], F32, tag="oht")
                nc.sync.dma_start(out=oht[:], in_=oh_ext.ap()[j])
                fixt = fx.tile([k_fix, B], F32, tag="fixt")
                nc.gpsimd.tensor_mul(fixt[:], selt[:], dpb[:])

                p_n = psp.tile([1, CHUNK], F32, tag="ps")
                for d in range(DT):
                    sq = sqp.tile([128, CHUNK], F32, tag="sq")
                    nc.scalar.activation(out=sq[:], in_=wch[:, d, :], func=AF.Square)
                    nc.tensor.matmul(
                        p_n[:], lhsT=oc[:], rhs=sq[:], start=(d == 0), stop=(d == DT - 1)
                    )
                lnn = rp.tile([1, CHUNK], F32, tag="lnn")
                nc.scalar.activation(out=lnn[:], in_=p_n[:], func=AF.Ln)
                rin = rp.tile([1, CHUNK], F32, tag="rin")
                nc.scalar.activation(out=rin[:], in_=lnn[:], func=AF.Exp, scale=-0.5)
                p_rb = psp.tile([128, CHUNK], F32, tag="ps")
                nc.tensor.matmul(p_rb[:], lhsT=orow[:], rhs=rin[:], start=True, stop=True)
                rb = rp.tile([128, CHUNK], F32, tag="rb")
                nc.scalar.copy(out=rb[:], in_=p_rb[:])

                outc = op.tile([128, BT, CHUNK], F32, tag="outc")
                for bt in range(BT):
                    p_m = psp.tile([128, CHUNK], F32, tag="ps")
                    for d in range(DT):
                        nc.tensor.matmul(
                            p_m[:],
                            lhsT=e64[:, d, bt * 128 : (bt + 1) * 128],
                            rhs=wch[:, d, :],
                            start=(d == 0),
                            stop=False,
                        )
                    nc.tensor.matmul(
                        p_m[:],
                        lhsT=fixt[:, bt * 128 : (bt + 1) * 128],
                        rhs=oht[:],
                        start=False,
                        stop=True,
                    )
                    nc.vector.tensor_mul(outc[:, bt, :], p_m[:], rb[:])
                nc.sync.dma_start(
                    out=out_ap[:, :, j * CHUNK : (j + 1) * CHUNK], in_=outc[:]
                )

    nsplit = _legalize_waits(nc)
    return nc


def _host_prep(embeddings, labels, class_weights, cs=CS):
    nchunk = cs // CHUNK
    embeddings = np.ascontiguousarray(np.asarray(embeddings, dtype=np.float32))
    labels = np.asarray(labels).astype(np.int64)
    class_weights = np.asarray(class_weights, dtype=np.float32)

    embt = np.ascontiguousarray(embeddings.T)                  # [D, B]
    wl = np.ascontiguousarray(class_weights[:, labels])        # [D, B]

    n_cores = class_weights.shape[1] // cs
    counts = np.zeros((n_cores, nchunk), dtype=np.int64)
    for l in labels:
        counts[l // cs, (l % cs) // CHUNK] += 1
    k_fix = max(16, int(counts.max()))

    sel = np.zeros((n_cores, nchunk, k_fix, B), dtype=np.float32)
    oh = np.zeros((n_cores, nchunk, k_fix, CHUNK), dtype=np.float32)
    slot = np.zeros((n_cores, nchunk), dtype=np.int64)
    for b, l in enumerate(labels):
        core = int(l) // cs
        j = (int(l) % cs) // CHUNK
        c_loc = (int(l) % cs) % CHUNK
        k = slot[core, j]
        slot[core, j] += 1
        sel[core, j, k, b] = 1.0
        oh[core, j, k, c_loc] = 1.0

    onesc = np.ones((128, 1), dtype=np.float32)
    onesr = np.ones((1, 128), dtype=np.float32)

    in_maps = []
    for core in range(n_cores):
        in_maps.append(
            {
                "w": np.ascontiguousarray(class_weights[:, core * cs : (core + 1) * cs]),
                "embT": embt,
                "wl": wl,
                "sel": sel[core],
                "oh": oh[core],
                "onesc": onesc,
                "onesr": onesr,
            }
        )
    return k_fix, in_maps


def kernel(embeddings, labels, class_weights, _trace=False):
    k_fix, in_maps = _host_prep(embeddings, labels, class_weights)
    nc = build(k_fix)
    res = run_bass_kernel_spmd(
        nc, in_maps, core_ids=list(range(N_CORES)), trace=_trace
    )
    out = np.concatenate([res.results[i]["out"] for i in range(N_CORES)], axis=1)
    if _trace:
        kernel.last_results = res
    return out.astype(np.float32)


# revision 9
# speedup vs baseline: 2.0781x; 2.0781x over previous
"""ArcFace logits kernel for 8 TRN2 NeuronCores (partial-FC tensor parallel).

logits = scale * where(one_hot(labels), cos(arccos(cosine)+m), cosine)
  cosine = normalize(emb) @ normalize(W)   [B=512, C=100000]

Sharding: W columns (and the [B, C] output) split across 8 cores, 12500
columns each; embeddings/labels broadcast. No collectives needed.

Per-core device pipeline, per 500-column chunk:
  - column norms via ones-vector matmul over W^2 (partition-axis reduce)
  - rsqrt broadcast to 128 partitions via K=1 ones matmul
  - main matmul: psum = (64 * emb_norm^T) @ W_raw, accumulated over D
  - margin fixup accumulated into the same psum as a K=n_slot one-hot
    matmul: host passes 0/1 selection masks (pure label-index plumbing),
    the device computes the per-row delta values from W[:, labels]
  - evict: out = psum * rsqrt_bcast  (one DVE tensor_tensor per tile)
"""

import math

import numpy as np

import concourse.bass as bass
import concourse.tile as tile
from concourse import mybir
from concourse.bass_utils import run_bass_kernel_spmd

N_CORES = 8
B = 512          # batch
D = 512          # embed dim
C = 100000       # num classes
CS = C // N_CORES          # 12500 columns per core
CHUNK = 500
DT = D // 128
BT = B // 128
SCALE = 64.0
MARGIN = 0.5
F32 = mybir.dt.float32
BF16 = mybir.dt.bfloat16
AF = mybir.ActivationFunctionType

_MAX_WAITS = 1


def _legalize_waits(nc, max_waits=_MAX_WAITS):
    """Split multi-wait instructions for this toolchain's codegen.

    The pinned neuronxcc rejects instructions carrying more than one sync
    wait ("Too many sync wait commands" in setupSyncWait). Tile's semaphore
    assignment can attach several waits to one instruction (tail drain,
    first matmul of a group). Hoist the overflow onto no-op instructions
    emitted just before, on the same engine — the engine blocks on those
    first, which is semantically identical.
    """
    n = 0
    for fn in nc.m.functions:
        for bb in fn.blocks:
            out = []
            for inst in bb.instructions:
                si = inst.sync_info
                if si is not None and si.on_wait and len(si.on_wait) > max_waits:
                    waits = list(si.on_wait)
                    keep = waits[-max_waits:]
                    over = waits[:-max_waits]
                    for i in range(0, len(over), max_waits):
                        nop = mybir.InstNoOp(
                            name=f"waitsplit_{n}",
                            sync_info=mybir.SyncInfo(
                                on_wait=over[i : i + max_waits], on_update=[]
                            ),
                            bass_nofuse=True,
                            engine=inst.engine,
                        )
                        n += 1
                        nc.register_instruction(nop)
                        out.append(nop)
                    inst.sync_info = mybir.SyncInfo(
                        on_wait=keep, on_update=list(si.on_update or [])
                    )
                out.append(inst)
            bb.instructions[:] = out
    return n


def build(k_fix, cs=CS, sq_bufs=6, w_bufs=3, out_bufs=3, ps_bufs=8):
    nchunk = cs // CHUNK
    nc = bass.Bass("TRN2", target_bir_lowering=False, debug=False, num_devices=N_CORES)
    w_ext = nc.declare_dram_parameter("w", [D, cs], F32, isOutput=False)
    embt_ext = nc.declare_dram_parameter("embT", [D, B], F32, isOutput=False)
    wl_ext = nc.declare_dram_parameter("wl", [D, B], F32, isOutput=False)
    sel_ext = nc.declare_dram_parameter("sel", [nchunk, k_fix, B], F32, isOutput=False)
    oh_ext = nc.declare_dram_parameter("oh", [nchunk, k_fix, CHUNK], BF16, isOutput=False)
    onesc_ext = nc.declare_dram_parameter("onesc", [128, 1], BF16, isOutput=False)
    onesr_ext = nc.declare_dram_parameter("onesr", [1, 128], F32, isOutput=False)
    onesrb_ext = nc.declare_dram_parameter("onesrb", [1, 128], BF16, isOutput=False)
    out_ext = nc.declare_dram_parameter("out", [B, cs], F32, isOutput=True)

    w_ap = w_ext.ap().rearrange("(a p) c -> p a c", p=128)      # [128, DT, cs]
    et_ap = embt_ext.ap().rearrange("(a p) b -> p a b", p=128)  # [128, DT, B]
    wl_ap = wl_ext.ap().rearrange("(a p) b -> p a b", p=128)
    out_ap = out_ext.ap().rearrange("(a p) c -> p a c", p=128)  # [128, BT, cs]

    cosm = math.cos(MARGIN)
    sinm = math.sin(MARGIN)

    with tile.TileContext(nc) as tc:
        with (
            tc.tile_pool(name="persist", bufs=1) as persist,
            tc.tile_pool(name="ps", bufs=ps_bufs, space="PSUM") as psp,
            tc.tile_pool(name="pro", bufs=3) as pro,
            tc.tile_pool(name="wp", bufs=w_bufs) as wp,
            tc.tile_pool(name="sqp", bufs=sq_bufs) as sqp,
            tc.tile_pool(name="rp", bufs=3) as rp,
            tc.tile_pool(name="op", bufs=out_bufs) as op,
            tc.tile_pool(name="fx", bufs=3) as fx,
        ):
            # ---- persistent tiles
            oc = persist.tile([128, 1], BF16)
            nc.sync.dma_start(out=oc[:], in_=onesc_ext.ap())
            orow = persist.tile([1, 128], F32)
            nc.sync.dma_start(out=orow[:], in_=onesr_ext.ap())
            orow_b = persist.tile([1, 128], BF16)
            nc.sync.dma_start(out=orow_b[:], in_=onesrb_ext.ap())
            et = persist.tile([128, DT, B], F32)
            nc.sync.dma_start(out=et[:], in_=et_ap)
            e64 = persist.tile([128, DT, B], F32)
            e64b = persist.tile([128, DT, B], BF16)
            ebc = persist.tile([128, B], F32)
            dpb = persist.tile([k_fix, B], F32)

            # ---- prologue: per-row margin delta from the gathered label
            # columns wl = W[:, labels], plus 64/||emb|| row scaling.
            wlt = pro.tile([128, DT, B], F32, bufs=1)
            nc.sync.dma_start(out=wlt[:], in_=wl_ap)
            ocf = persist.tile([128, 1], F32)
            nc.gpsimd.memset(ocf[:], 1.0)

            p_ne = psp.tile([1, B], F32, tag="ps")
            for d in range(DT):
                sq = pro.tile([128, B], F32)
                nc.scalar.activation(out=sq[:], in_=et[:, d, :], func=AF.Square)
                nc.tensor.matmul(
                    p_ne[:], lhsT=ocf[:], rhs=sq[:], start=(d == 0), stop=(d == DT - 1)
                )
            lne = pro.tile([1, B], F32, bufs=1)
            nc.scalar.activation(out=lne[:], in_=p_ne[:], func=AF.Ln)
            rinv_e = pro.tile([1, B], F32, bufs=1)
            nc.scalar.activation(out=rinv_e[:], in_=lne[:], func=AF.Exp, scale=-0.5)

            p_dot = psp.tile([1, B], F32, tag="ps")
            for d in range(DT):
                hd = pro.tile([128, B], F32)
                nc.vector.tensor_mul(hd[:], et[:, d, :], wlt[:, d, :])
                nc.tensor.matmul(
                    p_dot[:], lhsT=ocf[:], rhs=hd[:], start=(d == 0), stop=(d == DT - 1)
                )

            p_nw = psp.tile([1, B], F32, tag="ps")
            for d in range(DT):
                sqw = pro.tile([128, B], F32)
                nc.scalar.activation(out=sqw[:], in_=wlt[:, d, :], func=AF.Square)
                nc.tensor.matmul(
                    p_nw[:], lhsT=ocf[:], rhs=sqw[:], start=(d == 0), stop=(d == DT - 1)
                )
            lnw = pro.tile([1, B], F32, bufs=1)
            nc.scalar.activation(out=lnw[:], in_=p_nw[:], func=AF.Ln)
            rinv_w = pro.tile([1, B], F32, bufs=1)
            nc.scalar.activation(out=rinv_w[:], in_=lnw[:], func=AF.Exp, scale=-0.5)
            snw = pro.tile([1, B], F32, bufs=1)
            nc.scalar.activation(out=snw[:], in_=p_nw[:], func=AF.Sqrt)

            cos1 = pro.tile([1, B], F32, bufs=1)
            nc.vector.tensor_mul(cos1[:], p_dot[:], rinv_e[:])
            cosl = pro.tile([1, B], F32, bufs=1)
            nc.vector.tensor_mul(cosl[:], cos1[:], rinv_w[:])

            cos2 = pro.tile([1, B], F32)
            nc.scalar.activation(out=cos2[:], in_=cosl[:], func=AF.Square)
            om = pro.tile([1, B], F32)
            nc.vector.tensor_scalar(
                om[:], cos2[:], -1.0, 1.0, mybir.AluOpType.mult, mybir.AluOpType.add
            )
            sn = pro.tile([1, B], F32)
            nc.scalar.activation(out=sn[:], in_=om[:], func=AF.Sqrt)
            # delta64 = 64*(cos(th+m) - cos) = cos*64*(cosm-1) - sin*64*sinm
            t2 = pro.tile([1, B], F32)
            nc.scalar.mul(out=t2[:], in_=cosl[:], mul=SCALE * (cosm - 1.0))
            t3 = pro.tile([1, B], F32)
            nc.scalar.mul(out=t3[:], in_=sn[:], mul=-SCALE * sinm)
            d64 = pro.tile([1, B], F32)
            nc.vector.tensor_add(d64[:], t2[:], t3[:])
            # pre-divide by the rsqrt that the evict multiply will apply at
            # the label column: delta_pre = delta64 * ||w_label||
            dpre = pro.tile([1, B], F32, bufs=1)
            nc.vector.tensor_mul(dpre[:], d64[:], snw[:])

            # broadcast delta_pre to k_fix partitions (K=1 ones matmul)
            p_dpb = psp.tile([k_fix, B], F32, tag="ps")
            nc.tensor.matmul(
                p_dpb[:], lhsT=orow[:, :k_fix], rhs=dpre[:], start=True, stop=True
            )
            nc.scalar.copy(out=dpb[:], in_=p_dpb[:])

            # 64/||emb|| broadcast to 128 partitions, then scale embT
            rinv64 = pro.tile([1, B], F32, bufs=1)
            nc.scalar.mul(out=rinv64[:], in_=rinv_e[:], mul=SCALE)
            p_eb = psp.tile([128, B], F32, tag="ps")
            nc.tensor.matmul(p_eb[:], lhsT=orow[:], rhs=rinv64[:], start=True, stop=True)
            nc.scalar.copy(out=ebc[:], in_=p_eb[:])
            for d in range(DT):
                nc.vector.tensor_mul(e64[:, d, :], et[:, d, :], ebc[:])
            for d in range(DT):
                nc.vector.tensor_copy(e64b[:, d, :], e64[:, d, :])

            # ---- main loop over 500-column chunks
            for j in range(nchunk):
                wch = wp.tile([128, DT, CHUNK], F32, tag="wch")
                nc.sync.dma_start(
                    out=wch[:], in_=w_ap[:, :, j * CHUNK : (j + 1) * CHUNK]
                )
                selt = fx.tile([k_fix, B], F32, tag="selt")
                nc.sync.dma_start(out=selt[:], in_=sel_ext.ap()[j])
                oht = fx.tile([k_fix, CHUNK], BF16, tag="oht")
                nc.sync.dma_start(out=oht[:], in_=oh_ext.ap()[j])
                fixt = fx.tile([k_fix, B], BF16, tag="fixt")
                nc.gpsimd.tensor_mul(fixt[:], selt[:], dpb[:])

                wb = wp.tile([128, DT, CHUNK], BF16, tag="wb")
                for d in range(DT):
                    nc.vector.tensor_copy(wb[:, d, :], wch[:, d, :])
                p_n = psp.tile([1, CHUNK], F32, tag="ps")
                for d in range(DT):
                    sq = sqp.tile([128, CHUNK], BF16, tag="sq")
                    nc.scalar.activation(out=sq[:], in_=wb[:, d, :], func=AF.Square)
                    nc.tensor.matmul(
                        p_n[:], lhsT=oc[:], rhs=sq[:], start=(d == 0), stop=(d == DT - 1)
                    )
                lnn = rp.tile([1, CHUNK], F32, tag="lnn")
                nc.scalar.activation(out=lnn[:], in_=p_n[:], func=AF.Ln)
                rin = rp.tile([1, CHUNK], BF16, tag="rin")
                nc.scalar.activation(out=rin[:], in_=lnn[:], func=AF.Exp, scale=-0.5)
                p_rb = psp.tile([128, CHUNK], F32, tag="ps")
                nc.tensor.matmul(p_rb[:], lhsT=orow_b[:], rhs=rin[:], start=True, stop=True)
                rb = rp.tile([128, CHUNK], F32, tag="rb")
                nc.scalar.copy(out=rb[:], in_=p_rb[:])

                outc = op.tile([128, BT, CHUNK], F32, tag="outc")
                for bt in range(BT):
                    p_m = psp.tile([128, CHUNK], F32, tag="ps")
                    for d in range(DT):
                        nc.tensor.matmul(
                            p_m[:],
                            lhsT=e64b[:, d, bt * 128 : (bt + 1) * 128],
                            rhs=wb[:, d, :],
                            start=(d == 0),
                            stop=False,
                        )
                    nc.tensor.matmul(
                        p_m[:],
                        lhsT=fixt[:, bt * 128 : (bt + 1) * 128],
                        rhs=oht[:],
                        start=False,
                        stop=True,
                    )
                    nc.vector.tensor_mul(outc[:, bt, :], p_m[:], rb[:])
                nc.sync.dma_start(
                    out=out_ap[:, :, j * CHUNK : (j + 1) * CHUNK], in_=outc[:]
                )

    nsplit = _legalize_waits(nc)
    return nc


def _host_prep(embeddings, labels, class_weights, cs=CS):
    nchunk = cs // CHUNK
    embeddings = np.ascontiguousarray(np.asarray(embeddings, dtype=np.float32))
    labels = np.asarray(labels).astype(np.int64)
    class_weights = np.asarray(class_weights, dtype=np.float32)

    embt = np.ascontiguousarray(embeddings.T)                  # [D, B]
    wl = np.ascontiguousarray(class_weights[:, labels])        # [D, B]

    n_cores = class_weights.shape[1] // cs
    counts = np.zeros((n_cores, nchunk), dtype=np.int64)
    for l in labels:
        counts[l // cs, (l % cs) // CHUNK] += 1
    k_fix = max(16, int(counts.max()))

    sel = np.zeros((n_cores, nchunk, k_fix, B), dtype=np.float32)
    oh = np.zeros((n_cores, nchunk, k_fix, CHUNK), dtype=np.float32)
    slot = np.zeros((n_cores, nchunk), dtype=np.int64)
    for b, l in enumerate(labels):
        core = int(l) // cs
        j = (int(l) % cs) // CHUNK
        c_loc = (int(l) % cs) % CHUNK
        k = slot[core, j]
        slot[core, j] += 1
        sel[core, j, k, b] = 1.0
        oh[core, j, k, c_loc] = 1.0

    bf16 = mybir.dt.np(mybir.dt.bfloat16)
    onesc = np.ones((128, 1), dtype=bf16)
    onesr = np.ones((1, 128), dtype=np.float32)
    onesrb = np.ones((1, 128), dtype=bf16)
    oh = oh.astype(bf16)

    in_maps = []
    for core in range(n_cores):
        in_maps.append(
            {
                "w": np.ascontiguousarray(class_weights[:, core * cs : (core + 1) * cs]),
                "embT": embt,
                "wl": wl,
                "sel": sel[core],
                "oh": oh[core],
                "onesc": onesc,
                "onesr": onesr,
                "onesrb": onesrb,
            }
        )
    return k_fix, in_maps


def kernel(embeddings, labels, class_weights, _trace=False):
    k_fix, in_maps = _host_prep(embeddings, labels, class_weights)
    nc = build(k_fix)
    res = run_bass_kernel_spmd(
        nc, in_maps, core_ids=list(range(N_CORES)), trace=_trace
    )
    out = np.concatenate([res.results[i]["out"] for i in range(N_CORES)], axis=1)
    if _trace:
        kernel.last_results = res
    return out.astype(np.float32)


# revision 11
# speedup vs baseline: 2.7305x; 1.3139x over previous
"""ArcFace logits kernel for 8 TRN2 NeuronCores (partial-FC tensor parallel).

logits = scale * where(one_hot(labels), cos(arccos(cosine)+m), cosine)
  cosine = normalize(emb) @ normalize(W)   [B=512, C=100000]

Sharding: W columns (and the [B, C] output) split across 8 cores, 12500
columns each; embeddings/labels broadcast. No collectives needed.

Per-core device pipeline, per 500-column chunk:
  - column norms via ones-vector matmul over W^2 (partition-axis reduce)
  - rsqrt broadcast to 128 partitions via K=1 ones matmul
  - main matmul: psum = (64 * emb_norm^T) @ W_raw, accumulated over D
  - margin fixup accumulated into the same psum as a K=n_slot one-hot
    matmul: host passes 0/1 selection masks (pure label-index plumbing),
    the device computes the per-row delta values from W[:, labels]
  - evict: out = psum * rsqrt_bcast  (one DVE tensor_tensor per tile)
"""

import math

import numpy as np

import concourse.bass as bass
import concourse.tile as tile
from concourse import mybir
from concourse.bass_utils import run_bass_kernel_spmd

N_CORES = 8
B = 512          # batch
D = 512          # embed dim
C = 100000       # num classes
CS = C // N_CORES          # 12500 columns per core
CHUNK = 500
DT = D // 128
BT = B // 128
SCALE = 64.0
MARGIN = 0.5
F32 = mybir.dt.float32
BF16 = mybir.dt.bfloat16
AF = mybir.ActivationFunctionType

_MAX_WAITS = 1


def _legalize_waits(nc, max_waits=_MAX_WAITS):
    """Split multi-wait instructions for this toolchain's codegen.

    The pinned neuronxcc rejects instructions carrying more than one sync
    wait ("Too many sync wait commands" in setupSyncWait). Tile's semaphore
    assignment can attach several waits to one instruction (tail drain,
    first matmul of a group). Hoist the overflow onto no-op instructions
    emitted just before, on the same engine — the engine blocks on those
    first, which is semantically identical.
    """
    n = 0
    for fn in nc.m.functions:
        for bb in fn.blocks:
            out = []
            for inst in bb.instructions:
                si = inst.sync_info
                if si is not None and si.on_wait and len(si.on_wait) > max_waits:
                    waits = list(si.on_wait)
                    keep = waits[-max_waits:]
                    over = waits[:-max_waits]
                    for i in range(0, len(over), max_waits):
                        nop = mybir.InstNoOp(
                            name=f"waitsplit_{n}",
                            sync_info=mybir.SyncInfo(
                                on_wait=over[i : i + max_waits], on_update=[]
                            ),
                            bass_nofuse=True,
                            engine=inst.engine,
                        )
                        n += 1
                        nc.register_instruction(nop)
                        out.append(nop)
                    inst.sync_info = mybir.SyncInfo(
                        on_wait=keep, on_update=list(si.on_update or [])
                    )
                out.append(inst)
            bb.instructions[:] = out
    return n


def build(k_fix, cs=CS, sq_bufs=6, w_bufs=3, out_bufs=3, ps_bufs=5):
    nchunk = cs // CHUNK
    nc = bass.Bass("TRN2", target_bir_lowering=False, debug=False, num_devices=N_CORES)
    w_ext = nc.declare_dram_parameter("w", [D, cs], F32, isOutput=False)
    embt_ext = nc.declare_dram_parameter("embT", [D, B], F32, isOutput=False)
    wl_ext = nc.declare_dram_parameter("wl", [D, B], F32, isOutput=False)
    sel_ext = nc.declare_dram_parameter("sel", [nchunk, k_fix, B], F32, isOutput=False)
    oh_ext = nc.declare_dram_parameter("oh", [nchunk, k_fix, CHUNK], BF16, isOutput=False)
    onesc_ext = nc.declare_dram_parameter("onesc", [128, 1], BF16, isOutput=False)
    onesr_ext = nc.declare_dram_parameter("onesr", [1, 128], F32, isOutput=False)
    onesrb_ext = nc.declare_dram_parameter("onesrb", [1, 128], BF16, isOutput=False)
    out_ext = nc.declare_dram_parameter("out", [B, cs], F32, isOutput=True)

    w_ap = w_ext.ap().rearrange("(a p) c -> p a c", p=128)      # [128, DT, cs]
    et_ap = embt_ext.ap().rearrange("(a p) b -> p a b", p=128)  # [128, DT, B]
    wl_ap = wl_ext.ap().rearrange("(a p) b -> p a b", p=128)
    out_ap = out_ext.ap().rearrange("(a p) c -> p a c", p=128)  # [128, BT, cs]

    cosm = math.cos(MARGIN)
    sinm = math.sin(MARGIN)

    with tile.TileContext(nc) as tc:
        with (
            tc.tile_pool(name="persist", bufs=1) as persist,
            tc.tile_pool(name="ps", bufs=ps_bufs, space="PSUM") as psp,
            tc.tile_pool(name="pro", bufs=3) as pro,
            tc.tile_pool(name="wp", bufs=w_bufs) as wp,
            tc.tile_pool(name="sqp", bufs=sq_bufs) as sqp,
            tc.tile_pool(name="rp", bufs=3) as rp,
            tc.tile_pool(name="op", bufs=out_bufs) as op,
            tc.tile_pool(name="fx", bufs=3) as fx,
        ):
            # ---- persistent tiles
            oc = persist.tile([128, 1], BF16)
            nc.sync.dma_start(out=oc[:], in_=onesc_ext.ap())
            orow = persist.tile([1, 128], F32)
            nc.sync.dma_start(out=orow[:], in_=onesr_ext.ap())
            orow_b = persist.tile([1, 128], BF16)
            nc.sync.dma_start(out=orow_b[:], in_=onesrb_ext.ap())
            et = persist.tile([128, DT, B], F32)
            nc.sync.dma_start(out=et[:], in_=et_ap)
            e64 = persist.tile([128, DT, B], F32)
            e64b = persist.tile([128, DT, B], BF16)
            ebc = persist.tile([128, B], F32)
            dpb = persist.tile([k_fix, B], F32)

            # ---- prologue: per-row margin delta from the gathered label
            # columns wl = W[:, labels], plus 64/||emb|| row scaling.
            wlt = pro.tile([128, DT, B], F32, bufs=1)
            nc.sync.dma_start(out=wlt[:], in_=wl_ap)
            ocf = persist.tile([128, 1], F32)
            nc.gpsimd.memset(ocf[:], 1.0)

            p_ne = psp.tile([1, B], F32, tag="ps")
            for d in range(DT):
                sq = pro.tile([128, B], F32)
                nc.scalar.activation(out=sq[:], in_=et[:, d, :], func=AF.Square)
                nc.tensor.matmul(
                    p_ne[:], lhsT=ocf[:], rhs=sq[:], start=(d == 0), stop=(d == DT - 1)
                )
            lne = pro.tile([1, B], F32, bufs=1)
            nc.scalar.activation(out=lne[:], in_=p_ne[:], func=AF.Ln)
            rinv_e = pro.tile([1, B], F32, bufs=1)
            nc.scalar.activation(out=rinv_e[:], in_=lne[:], func=AF.Exp, scale=-0.5)

            p_dot = psp.tile([1, B], F32, tag="ps")
            for d in range(DT):
                hd = pro.tile([128, B], F32)
                nc.vector.tensor_mul(hd[:], et[:, d, :], wlt[:, d, :])
                nc.tensor.matmul(
                    p_dot[:], lhsT=ocf[:], rhs=hd[:], start=(d == 0), stop=(d == DT - 1)
                )

            p_nw = psp.tile([1, B], F32, tag="ps")
            for d in range(DT):
                sqw = pro.tile([128, B], F32)
                nc.scalar.activation(out=sqw[:], in_=wlt[:, d, :], func=AF.Square)
                nc.tensor.matmul(
                    p_nw[:], lhsT=ocf[:], rhs=sqw[:], start=(d == 0), stop=(d == DT - 1)
                )
            lnw = pro.tile([1, B], F32, bufs=1)
            nc.scalar.activation(out=lnw[:], in_=p_nw[:], func=AF.Ln)
            rinv_w = pro.tile([1, B], F32, bufs=1)
            nc.scalar.activation(out=rinv_w[:], in_=lnw[:], func=AF.Exp, scale=-0.5)
            snw = pro.tile([1, B], F32, bufs=1)
            nc.scalar.activation(out=snw[:], in_=p_nw[:], func=AF.Sqrt)

            cos1 = pro.tile([1, B], F32, bufs=1)
            nc.vector.tensor_mul(cos1[:], p_dot[:], rinv_e[:])
            cosl = pro.tile([1, B], F32, bufs=1)
            nc.vector.tensor_mul(cosl[:], cos1[:], rinv_w[:])

            cos2 = pro.tile([1, B], F32)
            nc.scalar.activation(out=cos2[:], in_=cosl[:], func=AF.Square)
            om = pro.tile([1, B], F32)
            nc.vector.tensor_scalar(
                om[:], cos2[:], -1.0, 1.0, mybir.AluOpType.mult, mybir.AluOpType.add
            )
            sn = pro.tile([1, B], F32)
            nc.scalar.activation(out=sn[:], in_=om[:], func=AF.Sqrt)
            # delta64 = 64*(cos(th+m) - cos) = cos*64*(cosm-1) - sin*64*sinm
            t2 = pro.tile([1, B], F32)
            nc.scalar.mul(out=t2[:], in_=cosl[:], mul=SCALE * (cosm - 1.0))
            t3 = pro.tile([1, B], F32)
            nc.scalar.mul(out=t3[:], in_=sn[:], mul=-SCALE * sinm)
            d64 = pro.tile([1, B], F32)
            nc.vector.tensor_add(d64[:], t2[:], t3[:])
            # pre-divide by the rsqrt that the evict multiply will apply at
            # the label column: delta_pre = delta64 * ||w_label||
            dpre = pro.tile([1, B], F32, bufs=1)
            nc.vector.tensor_mul(dpre[:], d64[:], snw[:])

            # broadcast delta_pre to k_fix partitions (K=1 ones matmul)
            p_dpb = psp.tile([k_fix, B], F32, tag="ps")
            nc.tensor.matmul(
                p_dpb[:], lhsT=orow[:, :k_fix], rhs=dpre[:], start=True, stop=True
            )
            nc.scalar.copy(out=dpb[:], in_=p_dpb[:])

            # 64/||emb|| broadcast to 128 partitions, then scale embT
            rinv64 = pro.tile([1, B], F32, bufs=1)
            nc.scalar.mul(out=rinv64[:], in_=rinv_e[:], mul=SCALE)
            p_eb = psp.tile([128, B], F32, tag="ps")
            nc.tensor.matmul(p_eb[:], lhsT=orow[:], rhs=rinv64[:], start=True, stop=True)
            nc.scalar.copy(out=ebc[:], in_=p_eb[:])
            for d in range(DT):
                nc.vector.tensor_mul(e64[:, d, :], et[:, d, :], ebc[:])
            for d in range(DT):
                nc.vector.tensor_copy(e64b[:, d, :], e64[:, d, :])

            # ---- main loop over 500-column chunks, with the column-norm
            # chain (squares -> ones-matmul -> Ln/Exp -> rsqrt broadcast)
            # software-pipelined one chunk ahead so the PE never waits on it.
            wch_t = {}
            rb_t = {}

            def emit_norm_stage(j):
                wch = wp.tile([128, DT, CHUNK], F32, tag="wch")
                nc.sync.dma_start(
                    out=wch[:], in_=w_ap[:, :, j * CHUNK : (j + 1) * CHUNK]
                )
                wch_t[j] = wch
                p_n = psp.tile([1, CHUNK], F32, tag="psn", bufs=1)
                for d in range(DT):
                    sq = sqp.tile([128, CHUNK], BF16, tag="sq")
                    nc.scalar.activation(out=sq[:], in_=wch[:, d, :], func=AF.Square)
                    nc.tensor.matmul(
                        p_n[:], lhsT=oc[:], rhs=sq[:], start=(d == 0), stop=(d == DT - 1)
                    )
                lnn = rp.tile([1, CHUNK], F32, tag="lnn")
                nc.scalar.activation(out=lnn[:], in_=p_n[:], func=AF.Ln)
                rin = rp.tile([1, CHUNK], BF16, tag="rin")
                nc.scalar.activation(out=rin[:], in_=lnn[:], func=AF.Exp, scale=-0.5)
                p_rb = psp.tile([128, CHUNK], F32, tag="psrb", bufs=2)
                nc.tensor.matmul(p_rb[:], lhsT=orow_b[:], rhs=rin[:], start=True, stop=True)
                rb = rp.tile([128, CHUNK], F32, tag="rb")
                nc.scalar.copy(out=rb[:], in_=p_rb[:])
                rb_t[j] = rb

            emit_norm_stage(0)
            for j in range(nchunk):
                wch = wch_t.pop(j)
                rb = rb_t.pop(j)
                wb = wp.tile([128, DT, CHUNK], BF16, tag="wb")
                for d in range(DT):
                    nc.vector.tensor_copy(wb[:, d, :], wch[:, d, :])
                selt = fx.tile([k_fix, B], F32, tag="selt")
                nc.sync.dma_start(out=selt[:], in_=sel_ext.ap()[j])
                oht = fx.tile([k_fix, CHUNK], BF16, tag="oht")
                nc.sync.dma_start(out=oht[:], in_=oh_ext.ap()[j])
                fixt = fx.tile([k_fix, B], BF16, tag="fixt")
                nc.gpsimd.tensor_mul(fixt[:], selt[:], dpb[:])

                outc = op.tile([128, BT, CHUNK], F32, tag="outc")
                for bt in range(BT):
                    p_m = psp.tile([128, CHUNK], F32, tag="ps")
                    for d in range(DT):
                        nc.tensor.matmul(
                            p_m[:],
                            lhsT=e64b[:, d, bt * 128 : (bt + 1) * 128],
                            rhs=wb[:, d, :],
                            start=(d == 0),
                            stop=False,
                        )
                    nc.tensor.matmul(
                        p_m[:],
                        lhsT=fixt[:, bt * 128 : (bt + 1) * 128],
                        rhs=oht[:],
                        start=False,
                        stop=True,
                    )
                    nc.vector.tensor_mul(outc[:, bt, :], p_m[:], rb[:])
                if j + 1 < nchunk:
                    emit_norm_stage(j + 1)
                nc.sync.dma_start(
                    out=out_ap[:, :, j * CHUNK : (j + 1) * CHUNK], in_=outc[:]
                )

    nsplit = _legalize_waits(nc)
    return nc


def _host_prep(embeddings, labels, class_weights, cs=CS):
    nchunk = cs // CHUNK
    embeddings = np.ascontiguousarray(np.asarray(embeddings, dtype=np.float32))
    labels = np.asarray(labels).astype(np.int64)
    class_weights = np.asarray(class_weights, dtype=np.float32)

    embt = np.ascontiguousarray(embeddings.T)                  # [D, B]
    wl = np.ascontiguousarray(class_weights[:, labels])        # [D, B]

    n_cores = class_weights.shape[1] // cs
    counts = np.zeros((n_cores, nchunk), dtype=np.int64)
    for l in labels:
        counts[l // cs, (l % cs) // CHUNK] += 1
    k_fix = max(16, int(counts.max()))

    sel = np.zeros((n_cores, nchunk, k_fix, B), dtype=np.float32)
    oh = np.zeros((n_cores, nchunk, k_fix, CHUNK), dtype=np.float32)
    slot = np.zeros((n_cores, nchunk), dtype=np.int64)
    for b, l in enumerate(labels):
        core = int(l) // cs
        j = (int(l) % cs) // CHUNK
        c_loc = (int(l) % cs) % CHUNK
        k = slot[core, j]
        slot[core, j] += 1
        sel[core, j, k, b] = 1.0
        oh[core, j, k, c_loc] = 1.0

    bf16 = mybir.dt.np(mybir.dt.bfloat16)
    onesc = np.ones((128, 1), dtype=bf16)
    onesr = np.ones((1, 128), dtype=np.float32)
    onesrb = np.ones((1, 128), dtype=bf16)
    oh = oh.astype(bf16)

    in_maps = []
    for core in range(n_cores):
        in_maps.append(
            {
                "w": np.ascontiguousarray(class_weights[:, core * cs : (core + 1) * cs]),
                "embT": embt,
                "wl": wl,
                "sel": sel[core],
                "oh": oh[core],
                "onesc": onesc,
                "onesr": onesr,
                "onesrb": onesrb,
            }
        )
    return k_fix, in_maps


def kernel(embeddings, labels, class_weights, _trace=False):
    k_fix, in_maps = _host_prep(embeddings, labels, class_weights)
    nc = build(k_fix)
    res = run_bass_kernel_spmd(
        nc, in_maps, core_ids=list(range(N_CORES)), trace=_trace
    )
    out = np.concatenate([res.results[i]["out"] for i in range(N_CORES)], axis=1)
    if _trace:
        kernel.last_results = res
    return out.astype(np.float32)
